# revision 11
# baseline (speedup 1.0000x reference)
"""Trainium2 Bass kernel for nn_Actor (gnn_message_passing).

Math (per batch b):
  k_mu = kv[..., :128], v_mu = kv[..., 128:256]
  rel[n,m]  = <k_mu[n], v_mu[m]> / sqrt(128)
  P[n,m,:]  = pos[n] - pos[m];  Pn = P / (||P|| + eps)
  out[n,:]  = 0.01 * tanh( sum_m Pn[n,m,:] * rel[n,m] )

Factored form used here (avoids materializing [N,N,3]):
  W[n,m]   = rel[n,m] / (sqrt(E)*||P[n,m]||)   (diagonal zeroed)
  out[n,d] = 0.01 * tanh( pos[n,d] * s[n] - (W @ pos)[n,d] ),  s[n] = sum_m W[n,m]

On-device pipeline per core (2 batches, data-parallel over B=16 across 8 cores):
  - atom index n maps to SBUF (partition p, tile t) as n = p*NT + t, so the
    12-byte-granule pos/out DMAs coalesce to 96B-per-partition packets
  - input DMAs are the first instruction on their queues (SWDGE f32->fp16
    cast for kv on gpsimd, pos on sync) so transfers start during program load
  - relT[m,n] via PE matmuls (fp16 operands, fp32 PSUM accum)
  - d2T[m,n] = |p_n - p_m|^2 via a K=18 fp16 split-precision matmul
  - ScalarE:  normt = Rsqrt(E*d2 + E*bias)  (raw InstActivation; measured
              4e-5 rel err on HW over the d2 range of this problem)
  - VectorE:  W = rel * max(normt, 0)  -- 2-uop custom op; the NaN-suppressing
              max launders rsqrt(negative) to W=0
  - PE:       P[4,N] accumulated IN-LOOP as column-tiled pairs
              (tile_position col groups; M=4 uses 1/32nd of the array, so two
              concurrent groups halve the streaming cost) -- keeping the PE
              dense also keeps the HAM clock gate at K=8/8 (2.4 GHz)
  - epilogue per batch immediately after its last P pack (b0's epilogue
    overlaps b1's tail groups); transpose P, combine, tanh, scale, DMA out
"""

import time

import numpy as np

import concourse.bass as bass
import concourse.bacc as bacc
import concourse.mybir as mybir
import concourse.tile as tile
import concourse.dve_ops as dve_ops
from concourse.bass_utils import run_bass_kernel_spmd
from concourse.dve_spec import Spec, Bin, AluOp, Src0, Src1, C0, C1, Zero, maxx, lower
from concourse.dve_uop import DveOpSpec
from concourse.masks import make_identity

F32 = mybir.dt.float32
F16 = mybir.dt.float16

B, N, CKV = 16, 1024, 259
E = 128
NCORES = 8
BPC = B // NCORES          # batches per core
NT = N // 128              # 128-row tiles per batch
KA = 18                    # augmented contraction size for the d2 matmul
NWARM = 18                 # PE warm-up matmuls (HAM clock-gate priming)
ACTION_SCALE = 0.01
D2_BIAS = 4e-6             # d2 is a true squared distance of the fp16-split
                           # points (pn2 built from a+b), so only PSUM rounding
                           # (~2e-6) can push d2 negative; 4e-6 clears it


def build_nc(stage=99):
    nc = bacc.Bacc("TRN2", target_bir_lowering=False, debug=False)
    kv_ext = nc.declare_dram_parameter("kv", [BPC, N, CKV], F32, isOutput=False)
    pos_ext = nc.declare_dram_parameter("positions", [BPC, N, 3], F32, isOutput=False)
    out_ext = nc.declare_dram_parameter("out", [BPC, N, 3], F32, isOutput=True)

    with tile.TileContext(nc) as tc:
        with (
            tc.tile_pool(name="const", bufs=1) as constp,
            tc.tile_pool(name="kv16", bufs=2) as kv16p,
            tc.tile_pool(name="kvT", bufs=2) as kvTp,
            tc.tile_pool(name="aug", bufs=2) as augp,
            tc.tile_pool(name="augT", bufs=2) as augTp,
            tc.tile_pool(name="norm", bufs=6) as normp,
            tc.tile_pool(name="wt", bufs=16) as wtp,
            tc.tile_pool(name="epi", bufs=2) as epip,
            tc.tile_pool(name="psrel", bufs=2, space="PSUM") as psrel,
            tc.tile_pool(name="psd2", bufs=2, space="PSUM") as psd2,
            tc.tile_pool(name="psP", bufs=1, space="PSUM") as psP,
        ):
            # ---- input DMAs first: transfers overlap program load ----
            # (one DMA per batch: splitting into t-halves measured +10us --
            # SWDGE descriptor overhead outweighs the earlier first-half)
            kv16s, posf = {}, {}
            for b in range(BPC):
                kv16s[b] = kv16p.tile(
                    [128, NT, 2 * E], F16, tag="kv16", name=f"kv16_{b}"
                )
                nc.gpsimd.dma_start(
                    out=kv16s[b][:, :, :],
                    in_=kv_ext[b].rearrange("(p t) c -> p t c", t=NT)[:, :, 0 : 2 * E],
                )
            for b in range(BPC):
                posf[b] = augp.tile([128, NT, 3], F32, tag="posf", name=f"posf{b}")
                nc.sync.dma_start(
                    out=posf[b][:, :, :],
                    in_=pos_ext[b].rearrange("(p t) d -> p t d", t=NT),
                )

            # ---- PE warm-up primer: dependency-free back-to-back matmuls ----
            # (zero-filled operands; issue during the DMA prologue and release
            # the HAM clock gate to 2.4 GHz before the real matmuls start)
            warm_in = constp.tile([128, 512], F16)
            nc.vector.memset(warm_in[:, :], 0.0)
            warm_ps = psrel.tile([128, 512], F32, tag="rel")
            for i in range(NWARM):
                nc.tensor.matmul(
                    warm_ps[:, :],
                    lhsT=warm_in[:, 0:128],
                    rhs=warm_in[:, :],
                    start=(i == 0),
                    stop=(i == NWARM - 1),
                )
            warm_sink = constp.tile([128, 1], F32)
            nc.vector.tensor_copy(warm_sink[:, :], warm_ps[:, 0:1])

            identity16 = constp.tile([128, 128], F16)
            make_identity(nc, identity16[:, :])
            identity32 = constp.tile([128, 128], F32)
            make_identity(nc, identity32[:, :])
            bias_tile = constp.tile([128, 1], F32)
            nc.gpsimd.memset(bias_tile[:, :], float(E) * D2_BIAS)
            # dummy activation: pulls the rsqrt ACT-table load to the head of
            # the kernel instead of blocking the first real rsqrt mid-loop
            act_warm = constp.tile([128, 1], F32)
            nc.scalar.activation(
                act_warm[:, :],
                bias_tile[:, 0:1],
                mybir.ActivationFunctionType.Abs_reciprocal_sqrt,
                bias=bias_tile[:, 0:1],
                scale=float(E),
            )

            pre_all = constp.tile([128, BPC, NT, 3], F32)

            # P accumulator for both batches: batch b accumulates t=0..3 into
            # column group 2b (PSUM partitions 64b..64b+3) and t=4..7 into
            # group 2b+1 (partitions 64b+32..64b+35)
            P_all = psP.tile([128, N], F32, name="P_all")

            kmuT, vmuT, A16T, B16T, X = {}, {}, {}, {}, {}

            def emit_prologue(b):
                # b0's prologue runs while DVE is otherwise idle; b1's runs
                # during b0's m-loop. ScalarE is reserved for the per-group
                # Rsqrt (it paces the loop), so drains go to DVE/GpSimd.
                ve = nc.vector if b == 0 else nc.gpsimd
                kv16 = kv16s[b]
                # ---- transpose k/v to [e, n] layout via PE ----
                # (XBAR DMA transpose measured ~1.2us per 128x128 tile --
                # ring-serialized 256B packets -- and starved the PE; the PE
                # path is ~10x faster and keeps the HAM clock gate warm)
                kmuT[b] = kvTp.tile([128, N], F16, tag="kmuT", name=f"kmuT{b}")
                vmuT[b] = kvTp.tile([128, N], F16, tag="vmuT", name=f"vmuT{b}")
                kT_ps = psd2.tile([128, N], F16, tag="d2")
                for t in range(NT):
                    nc.tensor.transpose(
                        kT_ps[:, 128 * t : 128 * (t + 1)],
                        kv16[:, t, 0:E],
                        identity16[:, :],
                    )
                nc.scalar.copy(kmuT[b][:, :], kT_ps[:, :])
                vT_ps = psd2.tile([128, N], F16, tag="d2")
                for t in range(NT):
                    nc.tensor.transpose(
                        vT_ps[:, 128 * t : 128 * (t + 1)],
                        kv16[:, t, E : 2 * E],
                        identity16[:, :],
                    )
                nc.scalar.copy(vmuT[b][:, :], vT_ps[:, :])

                # ---- build augmented position blocks (n-major, fp16) ----
                # moving rows A: [a(3), b(3), a(3), b(3), 1,1,1, pn2 h/m/l]
                # stationary rows Bm: [-2a(3), -2a(3), -2b(3), -2b(3), pm2 h/m/l, 1,1,1]
                pf = posf[b]
                A16 = augp.tile([128, NT, 64], F16, tag="A16")
                B16 = augp.tile([128, NT, 64], F16, tag="B16")
                sq3 = augp.tile([128, NT, 3], F32, tag="sq3")
                pn2 = augp.tile([128, NT, 1], F32, tag="pn2")
                t1 = augp.tile([128, NT, 1], F32, tag="t1")

                ve.tensor_copy(A16[:, :, 0:3], pf[:, :, :])      # a
                ve.tensor_sub(A16[:, :, 3:6], pf[:, :, :], A16[:, :, 0:3])
                ve.tensor_copy(A16[:, :, 6:9], A16[:, :, 0:3])
                ve.tensor_copy(A16[:, :, 9:12], A16[:, :, 3:6])
                ve.memset(A16[:, :, 12:15], 1.0)
                xs = augp.tile([128, NT, 3], F32, tag="xs")
                ve.tensor_add(xs[:, :, :], A16[:, :, 0:3], A16[:, :, 3:6])
                ve.tensor_mul(sq3[:, :, :], xs[:, :, :], xs[:, :, :])
                nc.vector.tensor_reduce(
                    out=pn2[:, :, :],
                    in_=sq3[:, :, :],
                    op=mybir.AluOpType.add,
                    axis=mybir.AxisListType.X,
                )
                ve.tensor_copy(A16[:, :, 15:16], pn2[:, :, :])   # h
                ve.tensor_sub(t1[:, :, :], pn2[:, :, :], A16[:, :, 15:16])
                ve.tensor_copy(A16[:, :, 16:17], t1[:, :, :])    # m
                ve.tensor_sub(t1[:, :, :], t1[:, :, :], A16[:, :, 16:17])
                ve.tensor_copy(A16[:, :, 17:18], t1[:, :, :])    # l

                ve.tensor_scalar_mul(B16[:, :, 0:3], A16[:, :, 0:3], -2.0)
                ve.tensor_copy(B16[:, :, 3:6], B16[:, :, 0:3])
                ve.tensor_scalar_mul(B16[:, :, 6:9], A16[:, :, 3:6], -2.0)
                ve.tensor_copy(B16[:, :, 9:12], B16[:, :, 6:9])
                ve.tensor_copy(B16[:, :, 12:15], A16[:, :, 15:18])
                ve.memset(B16[:, :, 15:18], 1.0)
                # replicate the KA aug rows at free offset 32: after the PE
                # transpose they land on partitions 32..49, feeding the second
                # row-group of the 2-way tile_position d2 packs
                ve.tensor_copy(A16[:, :, 32 : 32 + KA], A16[:, :, 0:KA])
                ve.tensor_copy(B16[:, :, 32 : 32 + KA], B16[:, :, 0:KA])

                X[b] = augp.tile([128, NT, 4], F16, tag="X", name=f"X{b}")
                ve.tensor_copy(X[b][:, :, 0:3], A16[:, :, 0:3])
                ve.memset(X[b][:, :, 3:4], 1.0)

                # ---- transpose aug blocks to [KA, N] via PE ----
                A_ps = psd2.tile([64, N], F16, tag="d2")
                for t in range(NT):
                    nc.tensor.transpose(
                        A_ps[0 : 32 + KA, 128 * t : 128 * (t + 1)],
                        A16[:, t, 0 : 32 + KA],
                        identity16[:, :],
                    )
                A16T[b] = augTp.tile([64, N], F16, tag="A16T", name=f"A16T{b}")
                nc.vector.tensor_copy(A16T[b][0 : 32 + KA, :], A_ps[0 : 32 + KA, :])

                B_ps = psd2.tile([64, N], F16, tag="d2")
                for t in range(NT):
                    nc.tensor.transpose(
                        B_ps[0 : 32 + KA, 128 * t : 128 * (t + 1)],
                        B16[:, t, 0 : 32 + KA],
                        identity16[:, :],
                    )
                B16T[b] = augTp.tile([64, N], F16, tag="B16T", name=f"B16T{b}")
                nc.vector.tensor_copy(B16T[b][0 : 32 + KA, :], B_ps[0 : 32 + KA, :])

            def emit_p_pack(b, tlo, thi):
                # single col group per batch (partitions 64b..64b+3); the
                # concurrent 4-group col-tiled form hangs the device
                for h in range(2):
                    cs = slice(512 * h, 512 * (h + 1))
                    for t in (tlo, thi):
                        g = 2 * b
                        nc.tensor.matmul(
                            P_all[32 * g : 32 * g + 4, cs],
                            lhsT=X[b][:, t, :],
                            rhs=wts[b][t][:, cs],
                            start=(t == 0),
                            stop=(t == 7),
                        )

            def emit_epi_pre(b):
                # P -> pre_all (no ScalarE ops, so it can overlap the other
                # batch's tail groups without thrashing the ACT table)
                Psb = epip.tile([128, N], F32, tag="Psb")
                nc.scalar.copy(Psb[64 * b : 64 * b + 36, :],
                               P_all[64 * b : 64 * b + 36, :])
                PT_ps = psrel.tile([128, NT * 4], F32, tag="rel")
                pb = 64 * b
                for c in range(NT):
                    nc.tensor.matmul(
                        PT_ps[:, 4 * c : 4 * (c + 1)],
                        lhsT=Psb[pb : pb + 4, 128 * c : 128 * (c + 1)],
                        rhs=identity32[pb : pb + 4, pb : pb + 4],
                        is_transpose=True,
                        start=True,
                        stop=True,
                        tile_position=(pb, 0),
                    )
                PT = epip.tile([128, NT, 4], F32, tag="PT")
                nc.vector.tensor_copy(
                    PT[:, :, :], PT_ps[:, :].rearrange("p (t f) -> p t f", f=4)
                )
                tmp = epip.tile([128, NT, 3], F32, tag="tmp")
                a0, a1 = bass.broadcast_tensor_aps(posf[b][:, :, :], PT[:, :, 3:4])
                nc.gpsimd.tensor_mul(tmp[:, :, :], a0, a1)
                nc.gpsimd.tensor_sub(pre_all[:, b, :, :], tmp[:, :, :], PT[:, :, 0:3])

            def emit_epi_post(b):
                act = epip.tile([128, NT, 3], F32, tag="act")
                nc.scalar.activation(
                    act[:, :, :],
                    pre_all[:, b, :, :],
                    mybir.ActivationFunctionType.Tanh,
                )
                actf = epip.tile([128, NT, 3], F32, tag="actf")
                nc.gpsimd.tensor_scalar_mul(actf[:, :, :], act[:, :, :], ACTION_SCALE)
                nc.sync.dma_start(
                    out=out_ext[b].rearrange("(p t) d -> p t d", t=NT),
                    in_=actf[:, :, :],
                )

            # ============ main loop: interleave both batches ============
            emit_prologue(0)
            pair_order = [(0, 0), (0, 1), (0, 2), ("pro1", None)]
            rest0 = [(0, t) for t in range(3, NT)]
            rest1 = [(1, t) for t in range(NT)]
            while rest0 or rest1:
                if rest0:
                    pair_order.append(rest0.pop(0))
                if rest1:
                    pair_order.append(rest1.pop(0))
            group_slots = [g for g in pair_order if g[0] != "pro1"]
            pos_of = {g: i for i, g in enumerate(group_slots)}
            packs_at = {}
            epi_at = {}
            for b in range(BPC):
                for t in range(4, NT):
                    packs_at.setdefault(pos_of[(b, t)] + 2, []).append(
                        (b, t - 4, t)
                    )
                epi_at.setdefault(pos_of[(b, NT - 1)] + 4, []).append(b)
            wts = {b: [None] * NT for b in range(BPC)}
            slot = 0
            for entry in pair_order:
                if entry[0] == "pro1":
                    emit_prologue(1)
                    continue
                for pk in packs_at.pop(slot, []):
                    emit_p_pack(*pk)
                for eb in epi_at.pop(slot, []):
                    emit_epi_pre(eb)
                b, t = entry
                # d2 on alternating PE row-groups (0/32): consecutive groups'
                # d2 matmuls land in different row strips, so the 64-deep PE
                # reorder window runs them concurrently -- without coupling
                # two PSUM tiles to one emission point (which serialized the
                # pipeline on the 2-buf psd2 ring)
                g32 = 32 * (t % 2)
                d2_ps = psd2.tile([128, N], F32, tag="d2")
                for h in range(2):
                    cs = slice(512 * h, 512 * (h + 1))
                    nc.tensor.matmul(
                        d2_ps[:, cs],
                        lhsT=B16T[b][g32 : g32 + KA, 128 * t : 128 * (t + 1)],
                        rhs=A16T[b][g32 : g32 + KA, cs],
                        start=True,
                        stop=True,
                        tile_position=(g32, 0),
                    )
                normt = normp.tile([128, N], F16)
                nc.scalar.activation(
                    normt[:, :],
                    d2_ps[:, :],
                    mybir.ActivationFunctionType.Abs_reciprocal_sqrt,
                    bias=bias_tile[:, 0:1],
                    scale=float(E),
                )
                wt = wtp.tile([128, N], F16)
                for h in range(2):
                    cs = slice(512 * h, 512 * (h + 1))
                    rel_ps = psrel.tile([128, 512], F32, tag="rel")
                    nc.tensor.matmul(
                        rel_ps[:, :],
                        lhsT=vmuT[b][:, 128 * t : 128 * (t + 1)],
                        rhs=kmuT[b][:, cs],
                        start=True,
                        stop=True,
                    )
                    nc.vector.tensor_mul(wt[:, cs], rel_ps[:, :], normt[:, cs])
                # overwrite the diagonal with 0 (robust even if the entry
                # were NaN/huge from a degenerate pair)
                nc.gpsimd.affine_select(
                    out=wt[:, 128 * t : 128 * (t + 1)],
                    in_=wt[:, 128 * t : 128 * (t + 1)],
                    compare_op=mybir.AluOpType.not_equal,
                    fill=0.0,
                    base=0,
                    pattern=[[-1, 128]],
                    channel_multiplier=1,
                )
                wts[b][t] = wt
                slot += 1

            # remaining P packs and epilogues (b0's pre-part overlaps b1's
            # tail; both tanh+scale+DMA go last for a single ACT-table switch)
            for sl in sorted(packs_at):
                for pk in packs_at[sl]:
                    emit_p_pack(*pk)
            for sl in sorted(epi_at):
                for eb in epi_at[sl]:
                    emit_epi_pre(eb)
            emit_epi_post(0)
            emit_epi_post(1)

    nc.compile()
    return nc


_NC_CACHE = {}


def _get_nc():
    if "nc" not in _NC_CACHE:
        _NC_CACHE["nc"] = build_nc()
    return _NC_CACHE["nc"]


def kernel(**inputs):
    kv = np.ascontiguousarray(np.asarray(inputs["kv"], dtype=np.float32))
    pos = np.ascontiguousarray(np.asarray(inputs["positions"], dtype=np.float32))
    assert kv.shape == (B, N, CKV) and pos.shape == (B, N, 3)
    nc = _get_nc()
    in_maps = [
        {
            "kv": kv[i * BPC : (i + 1) * BPC],
            "positions": pos[i * BPC : (i + 1) * BPC],
        }
        for i in range(NCORES)
    ]
    last_err = None
    for attempt in range(3):
        try:
            res = run_bass_kernel_spmd(nc, in_maps, core_ids=list(range(NCORES)))
            break
        except Exception as e:  # transient NRT device-state races between procs
            last_err = e
            if attempt == 2:
                raise
            time.sleep(2.0 * (attempt + 1))
    outs = res.results
    return np.concatenate([outs[i]["out"] for i in range(NCORES)], axis=0)


if __name__ == "__main__":
    rng = np.random.default_rng(0)
    kv = rng.standard_normal((B, N, CKV), dtype=np.float32)
    pos = rng.standard_normal((B, N, 3), dtype=np.float32)
    out = kernel(kv=kv, positions=pos)
    print("out", out.shape, out.dtype, float(np.abs(out).max()))

